# revision 27
# baseline (speedup 1.0000x reference)
"""Trainium2 Bass kernel for nn_CrossSelfDecoder (B=4,N=1024,D=1024,H=16,F=4096).

Sharding: 8 cores = (batch b in 0..3) x (head-half hh in 0..1). Each core
computes attention for its 8 heads over all 1024 positions of its batch.
Because the reference reshapes (B,H,N,Dp)->(B,N,D) without permuting heads
back, head-ownership makes row-ownership invariant: core (b,hh) owns rows
[512*hh, 512*hh+512) of batch b through the whole network.

Design:
- Host pre-transposes x1/x2 and pre-tiles all weights into bf16 DRAM
  layouts: zero device-side transposes, contiguous per-partition DMAs.
- All matmuls bf16 x bf16 with fp32 PSUM accumulate (PE at full rate).
- Attention: scores per (head, query-half) into [128,1024] PSUM tiles,
  exp merged to FD=1024 ACT calls, AV with a ones-column in V (M=65) so
  the softmax denominator falls out of the same matmul; denominator
  reciprocal via reciprocal_approx_fast + gpsimd partition_broadcast;
  the normalized output lands in the j-major transposed layout with 2
  coarse 4D-AP DVE ops per pair (no per-query scatter).
- LayerNorm after each attention (stats via 1/D-scaled ones matmuls,
  sqrt on ACT preloaded during the attention tail, rstd via
  reciprocal_approx_fast, gpsimd row broadcasts).
- One pairwise AllGather of the LN1 rows (bf16) overlapped with the
  own-row halves of the k2/v2 projections: keys are used in
  [own | partner] arrival order (softmax is key-permutation
  invariant), partner rows are recovered exactly as
  (block0 - own) + block1, and q2 reads the gathered blocks in global
  row order.
"""

import numpy as np
import ml_dtypes

import concourse.mybir as mybir
import concourse.tile as tile
from concourse import bacc
from concourse.bass_utils import run_bass_kernel_spmd

FP32 = mybir.dt.float32
BF16 = mybir.dt.bfloat16
AF = mybir.ActivationFunctionType
ALU = mybir.AluOpType

B, N, D, H, F = 4, 1024, 1024, 16, 4096
Dp = D // H           # 64
HPC = 8               # heads per core
PC = 128              # partition chunk
NF = 512              # free chunk (one psum bank of fp32)
KC = D // PC          # 8 contraction chunks
FT = F // PC          # 32 f-tiles
NB = 4                # LN/collective row bands of 128
EPS = 1e-5
BF = ml_dtypes.bfloat16

_CACHE = {}


def _build():
    nc = bacc.Bacc("TRN2", target_bir_lowering=False, debug=False,
                   num_devices=8)
    dram = {}
    specs = [
        ("x2t", [D, N], BF16), ("x1t", [D, N], BF16),
        ("x2own", [PC, KC * NF], BF16),
        ("wq", [4, PC, KC, PC], BF16), ("wk", [4, PC, KC, PC], BF16),
        ("wv", [KC, PC, NF], BF16),
        ("wq2", [4, PC, KC, PC], BF16), ("wk2", [4, PC, KC, PC], BF16),
        ("wv2", [KC, PC, NF], BF16),
        ("w1", [FT, PC, KC, PC], BF16), ("w2", [KC, PC, FT, PC], BF16),
        ("bqc", [PC, 4], FP32), ("bkc", [PC, 4], FP32),
        ("bq2c", [PC, 4], FP32), ("bk2c", [PC, 4], FP32),
        ("bvr", [1, NF], FP32), ("bv2r", [1, NF], FP32),
        ("b1c", [PC, FT], FP32), ("b2c", [PC, KC], FP32),
        ("gammac", [PC, KC], FP32), ("betac", [PC, KC], FP32),
    ]
    for nm, shp, dt in specs:
        dram[nm] = nc.dram_tensor(nm, shp, dt, kind="ExternalInput")
    y_out = nc.dram_tensor("y", [D, NF], FP32, kind="ExternalOutput")

    # collective staging: own LN1 rows out, both group blocks back
    ag_in = nc.dram_tensor("agin", [D, NF], BF16, kind="Internal")
    ag_out = nc.dram_tensor("agout", [2, 2, NF, NF], BF16,
                            kind="Internal")

    with tile.TileContext(nc) as tc:
        _emit(nc, tc, dram, ag_in, ag_out, y_out)
    nc.compile()
    return nc


def _proj_T(nc, sub, psp, w_dram, bias_cols, rhs_of, out_tiles, tag,
            nf_range=(0, 1)):
    """out[m][:, nf*512:...] = (W.T @ rhs + bias), T-domain."""
    for m in range(4):
        wt = sub.tile([PC, KC, PC], BF16, tag=f"w_{tag}", bufs=3,
                      name=f"w_{tag}{m}")
        nc.sync.dma_start(wt[:], w_dram.ap()[m])
        for nf in nf_range:
            ps = psp.tile([PC, NF], FP32, tag="proj", bufs=7,
                          name=f"proj_{tag}{m}_{nf}")
            for kc in range(KC):
                nc.tensor.matmul(ps[:], wt[:, kc, :], rhs_of(kc, nf),
                                 start=(kc == 0), stop=(kc == KC - 1))
            nc.scalar.activation(out_tiles[m][:, nf * NF:(nf + 1) * NF],
                                 ps[:], AF.Identity,
                                 bias=bias_cols[:, m:m + 1])


def _proj_v(nc, sub, psp, w_dram, bvB, rhs_of, v_tiles, tag,
            pc_range=(0, 8)):
    """v natural (keys x 512 own-head cols) + per-head ones column.
    v_tiles: 8 x (128, 520): head h data cols [65h,65h+64), col 65h+64=1."""
    wts = []
    for kc in range(KC):
        wt = sub.tile([PC, NF], BF16, tag=f"wv_{tag}", bufs=KC,
                      name=f"wv_{tag}{kc}")
        nc.sync.dma_start(wt[:], w_dram.ap()[kc])
        wts.append(wt)
    for pc in range(*pc_range):
        ps = psp.tile([PC, NF], FP32, tag="proj", bufs=7, name=f"v_{tag}{pc}")
        for kc in range(KC):
            nc.tensor.matmul(ps[:], rhs_of(kc, pc), wts[kc][:],
                             start=(kc == 0), stop=(kc == KC - 1))
        vt3 = v_tiles[pc][:].rearrange("p (h c) -> p h c", h=HPC)
        ps3 = ps[:].rearrange("p (h c) -> p h c", h=HPC)
        bb3 = bvB[:].rearrange("p (h c) -> p h c", h=HPC)
        nc.vector.tensor_tensor(vt3[:, :, 0:Dp], ps3, bb3, op=ALU.add)


def _attention(nc, tc, sub, psp, qT, kT, v_tiles, xT, tag,
               after_group=None):
    """Own-head attention; writes normalized output into xT [128, 4096]
    (j-major T-domain). Query columns are in natural order; the output
    of query n lands at partition 64*(n%2)+dp of j-block (n%16)//2 at
    free offset 64*hloc + (n%512)//16. after_group(g) is invoked after
    the two heads covering xT row band [128g, 128g+128) are emitted."""
    for hloc in range(HPC):
        t4, r64 = hloc // 2, Dp * (hloc % 2)
        for qh in range(2):
            pts = []
            for g in range(4):          # kc groups of 2
                sps = psp.tile([PC, 2 * NF], FP32, tag="S", bufs=2,
                               name=f"S_{tag}{hloc}_{qh}_{g}")
                for k2 in range(2):
                    kc = 2 * g + k2
                    nc.tensor.matmul(
                        sps[:, k2 * NF:(k2 + 1) * NF],
                        kT[t4][r64:r64 + Dp, kc * PC:(kc + 1) * PC],
                        qT[t4][r64:r64 + Dp, qh * NF:(qh + 1) * NF],
                        start=True, stop=True)
                pt = sub.tile([PC, 2 * NF], BF16, tag="PT", bufs=8,
                              name=f"PT_{tag}{hloc}_{qh}_{g}")
                nc.scalar.activation(pt[:], sps[:], AF.Exp)
                pts.append(pt)
            ops = psp.tile([Dp + 1, NF], FP32, tag="O", bufs=4,
                           name=f"O_{tag}{hloc}_{qh}")
            for kc in range(KC):
                nc.tensor.matmul(
                    ops[:], v_tiles[kc][:, 65 * hloc:65 * hloc + 65],
                    pts[kc // 2][:, (kc % 2) * NF:(kc % 2 + 1) * NF],
                    start=(kc == 0), stop=(kc == KC - 1))
            drow = sub.tile([1, NF], FP32, tag="drow", bufs=4,
                            name=f"dr_{tag}{hloc}_{qh}")
            nc.vector.tensor_copy(drow[:], ops[Dp:Dp + 1, :])
            rrow = sub.tile([1, NF], FP32, tag="rrow", bufs=4,
                            name=f"rr_{tag}{hloc}_{qh}")
            nc.vector.reciprocal_approx_fast(rrow[:], drow[:])
            rb = sub.tile([Dp, NF], FP32, tag="rb", bufs=4,
                          name=f"rb_{tag}{hloc}_{qh}")
            nc.gpsimd.partition_broadcast(rb[:], rrow[:])
            # normalized scatter, 2 coarse ops (mm = n%2):
            # src col (within qh half) = 128wa + 16wb + 2j + mm
            # dst free = j*512 + 64hloc + 32qh + 8wa + wb
            toff = 8 * hloc + 4 * qh
            dst4 = xT.rearrange("p (j t wb) -> p j t wb", j=8, t=64, wb=8)
            for mm in range(2):
                dst = dst4[Dp * mm:Dp * mm + Dp, :, toff:toff + 4, :]
                src = ops[0:Dp, :].rearrange(
                    "d (wa wb j m) -> d m j wa wb",
                    wa=4, wb=8, j=8, m=2)[:, mm]
                srb = rb[:].rearrange(
                    "d (wa wb j m) -> d m j wa wb",
                    wa=4, wb=8, j=8, m=2)[:, mm]
                nc.vector.tensor_tensor(dst, src, srb, op=ALU.mult)
        if after_group is not None and hloc % 2 == 1:
            after_group(hloc // 2)


def _ln_full(nc, tc, sub, xT, write_out, c, tag, warm_n=40):
    """LayerNorm of all 512 rows of xT [128, 4096] over the feature
    axis. write_out(j, src_tile) stores the j-th [128, 512] result."""
    with tc.tile_pool(name=f"ln_{tag}", space="PSUM", bufs=1) as psp:
        lnr = psp.tile([33, NF], FP32, tag="lnr", bufs=1,
                       name=f"lnr_{tag}")
        sq = None
        for j in range(KC):
            xj = xT[:, j * NF:(j + 1) * NF]
            nc.tensor.matmul(lnr[0:1, :], c["onesd"][:], xj,
                             start=(j == 0), stop=(j == KC - 1))
            sq = sub.tile([PC, NF], BF16, tag="sq", bufs=3,
                          name=f"sq_{tag}{j}")
            nc.vector.tensor_tensor(sq[:], xj, xj, op=ALU.mult)
            nc.tensor.matmul(lnr[32:33, :], c["onesd"][:], sq[:],
                             start=(j == 0), stop=(j == KC - 1))
        # keep the PE clock-gate warm through the rows/apply window
        wps = psp.tile([1, NF], FP32, tag="lnwarm", bufs=1,
                       name=f"lnwarm_{tag}")
        for i in range(warm_n):
            nc.tensor.matmul(wps[:], c["onesd"][:], sq[:],
                             start=(i == 0), stop=(i == warm_n - 1))
        # preload the sqrt table while the attention tail drains
        wsq = sub.tile([1, 8], FP32, tag="lrow", bufs=8, name=f"wsq_{tag}")
        nc.gpsimd.memset(wsq[:], 1.0)
        nc.scalar.activation(wsq[:], wsq[:], AF.Sqrt)
        mu = sub.tile([1, NF], FP32, tag="lrow", bufs=8, name=f"mu_{tag}")
        nc.vector.tensor_copy(mu[:], lnr[0:1, :])
        mub = sub.tile([PC, NF], FP32, tag="lnb", bufs=2, name=f"mub_{tag}")
        nc.gpsimd.partition_broadcast(mub[:], mu[:])
        mu2 = sub.tile([1, NF], FP32, tag="lrow", bufs=8, name=f"mu2_{tag}")
        nc.vector.tensor_tensor(mu2[:], mu[:], mu[:], op=ALU.mult)
        var = sub.tile([1, NF], FP32, tag="lrow", bufs=8, name=f"var_{tag}")
        nc.vector.tensor_tensor(var[:], lnr[32:33, :], mu2[:],
                                op=ALU.subtract)
        std = sub.tile([1, NF], FP32, tag="lrow", bufs=8, name=f"std_{tag}")
        nc.scalar.activation(std[:], var[:], AF.Sqrt, bias=c["eps_sc"][:])
        rstd = sub.tile([1, NF], FP32, tag="lrow", bufs=8,
                        name=f"rstd_{tag}")
        nc.vector.reciprocal_approx_fast(rstd[:], std[:])
        rstdb = sub.tile([PC, NF], FP32, tag="lnb", bufs=2,
                         name=f"rsb_{tag}")
        nc.gpsimd.partition_broadcast(rstdb[:], rstd[:])
        for j in range(KC):
            xj = xT[:, j * NF:(j + 1) * NF]
            t1 = sub.tile([PC, NF], BF16, tag="lntmp", bufs=3,
                          name=f"lt_{tag}{j}")
            nc.vector.tensor_tensor(t1[:], xj, mub[:], op=ALU.subtract)
            t2 = sub.tile([PC, NF], BF16, tag="lntmp2", bufs=3,
                          name=f"l2_{tag}{j}")
            nc.vector.tensor_tensor(t2[:], t1[:], rstdb[:], op=ALU.mult)
            write_out(j, t2)


def _ln_band(nc, sub, psp, xT, g, write_out, c, tag):
    """LayerNorm of xT row band [128g, 128g+128) (local rows), over the
    feature axis (partitions x 8 j-blocks). write_out(j, src_tile)
    stores the [128, 128] result for j-block j."""
    lnr = psp.tile([33, PC], FP32, tag="lnr", bufs=2, name=f"lnr_{tag}{g}")
    s0, s1 = lnr[0:1, :], lnr[32:33, :]
    for j in range(KC):
        xj = xT[:, j * NF + PC * g:j * NF + PC * g + PC]
        nc.tensor.matmul(s0, c["onesd"][:], xj,
                         start=(j == 0), stop=(j == KC - 1))
        sq = sub.tile([PC, PC], BF16, tag="sq", bufs=4,
                      name=f"sq_{tag}{g}_{j}")
        nc.vector.tensor_tensor(sq[:], xj, xj, op=ALU.mult)
        nc.tensor.matmul(s1, c["onesd"][:], sq[:],
                         start=(j == 0), stop=(j == KC - 1))
    # s0 = mean, s1 = E[x^2] (stat matmul ones are pre-scaled by 1/D)
    mu = sub.tile([1, PC], FP32, tag="lrow", bufs=8, name=f"mu_{tag}{g}")
    nc.vector.tensor_copy(mu[:], s0)
    mu2 = sub.tile([1, PC], FP32, tag="lrow", bufs=8, name=f"mu2_{tag}{g}")
    nc.vector.tensor_tensor(mu2[:], mu[:], mu[:], op=ALU.mult)
    var = sub.tile([1, PC], FP32, tag="lrow", bufs=8, name=f"var_{tag}{g}")
    nc.vector.tensor_tensor(var[:], s1, mu2[:], op=ALU.subtract)
    std = sub.tile([1, PC], FP32, tag="lrow", bufs=8, name=f"std_{tag}{g}")
    nc.scalar.activation(std[:], var[:], AF.Sqrt, bias=c["eps_sc"][:])
    rstd = sub.tile([1, PC], FP32, tag="lrow", bufs=8, name=f"rstd_{tag}{g}")
    nc.vector.reciprocal_approx_fast(rstd[:], std[:])
    mub = sub.tile([PC, PC], FP32, tag="lnb", bufs=4, name=f"mub_{tag}{g}")
    nc.gpsimd.partition_broadcast(mub[:], mu[:])
    rstdb = sub.tile([PC, PC], FP32, tag="lnb", bufs=4, name=f"rsb_{tag}{g}")
    nc.gpsimd.partition_broadcast(rstdb[:], rstd[:])
    for j in range(KC):
        xj = xT[:, j * NF + PC * g:j * NF + PC * g + PC]
        t1 = sub.tile([PC, PC], BF16, tag="lntmp", bufs=3,
                      name=f"lt_{tag}{g}_{j}")
        nc.vector.tensor_tensor(t1[:], xj, mub[:], op=ALU.subtract)
        t2 = sub.tile([PC, PC], BF16, tag="lntmp2", bufs=3,
                      name=f"l2_{tag}{g}_{j}")
        nc.vector.tensor_tensor(t2[:], t1[:], rstdb[:], op=ALU.mult)
        write_out(j, t2)


def _emit(nc, tc, dram, ag_in, ag_out, y_out):
    with tc.tile_pool(name="persist", bufs=1) as pp:
        def bias_tile(name):
            shp = list(dram[name].shape)
            return pp.tile(shp, FP32, tag=f"bt_{name}", name=f"bt_{name}")

        bias_names = ("bqc", "bkc", "bq2c", "bk2c", "b1c", "b2c",
                      "gammac", "betac")
        c = {}
        for nm in bias_names:
            c[nm] = bias_tile(nm)
        bvr = bias_tile("bvr")
        bv2r = bias_tile("bv2r")

        def load_biases():
            for nm in bias_names:
                nc.sync.dma_start(c[nm][:], dram[nm].ap())
            nc.sync.dma_start(bvr[:], dram["bvr"].ap())
            nc.sync.dma_start(bv2r[:], dram["bv2r"].ap())

        onesd = pp.tile([PC, 1], BF16, tag="onesd")
        nc.gpsimd.memset(onesd[:], 1.0 / D)
        c["onesd"] = onesd
        eps_sc = pp.tile([1, 1], FP32, tag="eps_sc")
        nc.gpsimd.memset(eps_sc[:], EPS)
        c["eps_sc"] = eps_sc

        bvB = pp.tile([PC, NF], FP32, tag="bvB")
        bv2B = pp.tile([PC, NF], FP32, tag="bv2B")

        # table warm-up: preload the exp set during initial DMAs
        warm = pp.tile([1, 8], FP32, tag="warm")
        nc.gpsimd.memset(warm[:], 1.0)
        nc.scalar.activation(warm[:], warm[:], AF.Exp)
        # PE warm-up: keep the HAM activity window busy while the first
        # input tiles stream in, so real matmuls start at full clock
        wmm = pp.tile([PC, NF], BF16, tag="wmm")
        nc.gpsimd.memset(wmm[:], 0.0)
        with tc.tile_pool(name="warmps", space="PSUM", bufs=1) as wps:
            wp = wps.tile([1, NF], FP32, tag="warmp", bufs=1)
            for i in range(40):
                nc.tensor.matmul(wp[:], onesd[:], wmm[:],
                                 start=(i == 0), stop=(i == 39))

        # cross-stage persistents
        nTo = pp.tile([PC, KC * NF], BF16, tag="nTo")
        n3T = pp.tile([PC, KC * NF], BF16, tag="n3T")

        # ---- stage 1 ----
        with tc.tile_pool(name="st1", bufs=1) as sub:
            x2own = sub.tile([PC, KC * NF], BF16, tag="x2own")
            qT = [sub.tile([PC, N], BF16, tag="qT", bufs=4, name=f"qT{i}")
                  for i in range(4)]
            kT = [sub.tile([PC, N], BF16, tag="kT", bufs=4, name=f"kT{i}")
                  for i in range(4)]
            vt = [sub.tile([PC, 65 * HPC], BF16, tag="vt", bufs=KC,
                           name=f"vt{i}") for i in range(KC)]
            xT = sub.tile([PC, KC * NF], BF16, tag="xT")

            with tc.tile_pool(name="s1x", bufs=1) as subx:
                x2T = [subx.tile([PC, N], BF16, tag="x2T", bufs=KC,
                                 name=f"x2T{i}") for i in range(KC)]
                for j in range(KC):
                    nc.sync.dma_start(
                        x2T[j][:], dram["x2t"].ap()[j * PC:(j + 1) * PC])
                load_biases()
                nc.gpsimd.partition_broadcast(bvB[:], bvr[:])
                nc.gpsimd.partition_broadcast(bv2B[:], bv2r[:])
                x1T = [subx.tile([PC, N], BF16, tag="x1T", bufs=KC,
                                 name=f"x1T{i}") for i in range(KC)]
                with tc.tile_pool(name="s1p", space="PSUM", bufs=1) as psp:
                    _proj_T(nc, subx, psp, dram["wq"], c["bqc"],
                            lambda kc, nf: x2T[kc][:, nf * NF:(nf + 1) * NF],
                            qT, "q")
                    for j in range(KC):
                        nc.sync.dma_start(
                            x1T[j][:],
                            dram["x1t"].ap()[j * PC:(j + 1) * PC])
                    nc.sync.dma_start(x2own[:], dram["x2own"].ap())
                    for i in range(KC):
                        v3 = vt[i][:].rearrange("p (h c) -> p h c", h=HPC)
                        nc.gpsimd.memset(v3[:, :, Dp:Dp + 1].squeeze(2), 1.0)
                    _proj_T(nc, subx, psp, dram["wk"], c["bkc"],
                            lambda kc, nf: x1T[kc][:, nf * NF:(nf + 1) * NF],
                            kT, "k")
                    _proj_v(nc, subx, psp, dram["wv"], bvB,
                            lambda kc, pc: x1T[kc][:, pc * PC:(pc + 1) * PC],
                            vt, "v1")

            with tc.tile_pool(name="s1a", space="PSUM", bufs=1) as psp:
                def after_group1(g):
                    # residual for band g (pure DVE, off the exp table)
                    bnd = xT[:].rearrange("p (j r) -> p j r", j=KC)[
                        :, :, PC * g:PC * g + PC]
                    x2b = x2own[:].rearrange("p (j r) -> p j r", j=KC)[
                        :, :, PC * g:PC * g + PC]
                    nc.vector.tensor_tensor(bnd, bnd, x2b, op=ALU.add)

                _attention(nc, tc, sub, psp, qT, kT, vt, xT[:], "x",
                           after_group=after_group1)

            def ln1_out(j, t2):
                nc.scalar.activation(
                    nTo[:, j * NF:(j + 1) * NF], t2[:], AF.Identity,
                    bias=c["betac"][:, j:j + 1],
                    scale=c["gammac"][:, j:j + 1])
                nc.sync.dma_start(ag_in.ap()[j * PC:(j + 1) * PC],
                                  nTo[:, j * NF:(j + 1) * NF])

            _ln_full(nc, tc, sub, xT[:], ln1_out, c, "ln1")
            for h in range(2):
                nc.gpsimd.collective_compute(
                    "AllGather", ALU.bypass,
                    replica_groups=[[0, 1], [2, 3], [4, 5], [6, 7]],
                    ins=[ag_in.ap()[h * NF:(h + 1) * NF]],
                    outs=[ag_out.ap()[h]])

        w1pre = [pp.tile([PC, KC, PC], BF16, tag="w1pre", bufs=8,
                         name=f"w1pre{i}") for i in range(8)]

        # ---- stage 2 ----
        # keys are used in arrival order [own rows | partner rows]
        # (softmax is key-permutation invariant); queries need global
        # order, which nTg (both gathered blocks) provides uniformly.
        with tc.tile_pool(name="st2", bufs=1) as sub:
            for f in range(8):
                nc.sync.dma_start(w1pre[f][:], dram["w1"].ap()[f])
            q2T = [sub.tile([PC, N], BF16, tag="q2T", bufs=4,
                            name=f"q2T{i}") for i in range(4)]
            k2T = [sub.tile([PC, N], BF16, tag="k2T", bufs=4,
                            name=f"k2T{i}") for i in range(4)]
            v2t = [sub.tile([PC, 65 * HPC], BF16, tag="v2t", bufs=KC,
                            name=f"v2t{i}") for i in range(KC)]
            x3T = sub.tile([PC, KC * NF], BF16, tag="x3T")
            for i in range(KC):
                v3 = v2t[i][:].rearrange("p (h c) -> p h c", h=HPC)
                nc.gpsimd.memset(v3[:, :, Dp:Dp + 1].squeeze(2), 1.0)

            with tc.tile_pool(name="s2p", space="PSUM", bufs=1) as psp:
                # own-row halves of k2/v2 run from nTo while the
                # AllGather is in flight
                _proj_v(nc, sub, psp, dram["wv2"], bv2B,
                        lambda kc, pc: nTo[:, kc * NF + pc * PC:
                                           kc * NF + (pc + 1) * PC],
                        v2t, "v2o", pc_range=(0, 4))
                k2w = [sub.tile([PC, KC, PC], BF16, tag="w_k2", bufs=4,
                                name=f"wk2_{m}") for m in range(4)]
                for m in range(4):
                    nc.sync.dma_start(k2w[m][:], dram["wk2"].ap()[m])
                for m in range(4):
                    ps = psp.tile([PC, NF], FP32, tag="proj", bufs=7,
                                  name=f"k2o_{m}")
                    for kc in range(KC):
                        nc.tensor.matmul(
                            ps[:], k2w[m][:, kc, :],
                            nTo[:, kc * NF:kc * NF + NF],
                            start=(kc == 0), stop=(kc == KC - 1))
                    nc.scalar.activation(k2T[m][:, 0:NF], ps[:],
                                         AF.Identity,
                                         bias=c["bk2c"][:, m:m + 1])

                wp2 = psp.tile([1, NF], FP32, tag="cwarm", bufs=1,
                               name="cwarm")
                for i in range(88):
                    nc.tensor.matmul(wp2[:], c["onesd"][:], k2T[0][:, 0:NF],
                                     start=(i == 0), stop=(i == 87))

                # gathered blocks (global row order) + exact partner
                # recovery: partner = (block0 - own) + block1
                nTg = [sub.tile([PC, N], BF16, tag="nTg", bufs=KC,
                                name=f"nTg{i}") for i in range(KC)]
                for j in range(KC):
                    for r in range(2):
                        nc.sync.dma_start(
                            nTg[j][:, r * NF:(r + 1) * NF],
                            ag_out.ap()[j // 4, r,
                                        (j % 4) * PC:(j % 4 + 1) * PC])
                nTp = [sub.tile([PC, NF], BF16, tag="nTp", bufs=KC,
                                name=f"nTp{i}") for i in range(KC)]
                for j in range(KC):
                    tdif = sub.tile([PC, NF], FP32, tag="tdif", bufs=4,
                                    name=f"tdif{j}")
                    nc.vector.tensor_tensor(
                        tdif[:], nTg[j][:, 0:NF],
                        nTo[:, j * NF:(j + 1) * NF], op=ALU.subtract)
                    nc.vector.tensor_tensor(
                        nTp[j][:], tdif[:], nTg[j][:, NF:N], op=ALU.add)

                _proj_v(nc, sub, psp, dram["wv2"], bv2B,
                        lambda kc, pc: nTp[kc][:, (pc - 4) * PC:
                                               (pc - 3) * PC],
                        v2t, "v2p", pc_range=(4, 8))
                for m in range(4):
                    ps = psp.tile([PC, NF], FP32, tag="proj", bufs=7,
                                  name=f"k2p_{m}")
                    for kc in range(KC):
                        nc.tensor.matmul(
                            ps[:], k2w[m][:, kc, :], nTp[kc][:],
                            start=(kc == 0), stop=(kc == KC - 1))
                    nc.scalar.activation(k2T[m][:, NF:N], ps[:],
                                         AF.Identity,
                                         bias=c["bk2c"][:, m:m + 1])
                _proj_T(nc, sub, psp, dram["wq2"], c["bq2c"],
                        lambda kc, nf: nTg[kc][:, nf * NF:(nf + 1) * NF],
                        q2T, "q2")

            with tc.tile_pool(name="s2a", space="PSUM", bufs=1) as psp:
                def after_group2(g):
                    bnd = x3T[:].rearrange("p (j r) -> p j r", j=KC)[
                        :, :, PC * g:PC * g + PC]
                    nob = nTo[:].rearrange("p (j r) -> p j r", j=KC)[
                        :, :, PC * g:PC * g + PC]
                    nc.vector.tensor_tensor(bnd, bnd, nob, op=ALU.add)

                _attention(nc, tc, sub, psp, q2T, k2T, v2t, x3T[:], "y",
                           after_group=after_group2)

            def ln2_out(j, t2):
                nc.scalar.activation(
                    n3T[:, j * NF:(j + 1) * NF], t2[:], AF.Identity,
                    bias=c["betac"][:, j:j + 1],
                    scale=c["gammac"][:, j:j + 1])

            _ln_full(nc, tc, sub, x3T[:], ln2_out, c, "ln2")

        # ---- stage 3: MLP ----
        with tc.tile_pool(name="s3", bufs=1) as sub:
            hT = [sub.tile([PC, NF], BF16, tag="hT", bufs=FT,
                           name=f"hT{i}") for i in range(FT)]
            with tc.tile_pool(name="s3p", space="PSUM", bufs=1) as psp:
                for f in range(FT):
                    if f < 8:
                        wt = w1pre[f]
                    else:
                        wt = sub.tile([PC, KC, PC], BF16, tag="w1t", bufs=4,
                                      name=f"w1t{f}")
                        nc.sync.dma_start(wt[:], dram["w1"].ap()[f])
                    ps = psp.tile([PC, NF], FP32, tag="mlp", bufs=8,
                                  name=f"h{f}")
                    for kc in range(KC):
                        nc.tensor.matmul(
                            ps[:], wt[:, kc, :],
                            n3T[:, kc * NF:(kc + 1) * NF],
                            start=(kc == 0), stop=(kc == KC - 1))
                    nc.scalar.activation(hT[f][:], ps[:], AF.Gelu,
                                         bias=c["b1c"][:, f:f + 1])
                for d in range(KC):
                    w2t = sub.tile([PC, FT, PC], BF16, tag="w2t", bufs=2,
                                   name=f"w2t{d}")
                    nc.sync.dma_start(w2t[:], dram["w2"].ap()[d])
                    ps = psp.tile([PC, NF], FP32, tag="mlp", bufs=8,
                                  name=f"yp{d}")
                    for f in range(FT):
                        nc.tensor.matmul(ps[:], w2t[:, f, :], hT[f][:],
                                         start=(f == 0), stop=(f == FT - 1))
                    yt = sub.tile([PC, NF], FP32, tag="yT", bufs=4,
                                  name=f"yT{d}")
                    nc.vector.scalar_tensor_tensor(
                        yt[:], ps[:], c["b2c"][:, d:d + 1],
                        n3T[:, d * NF:(d + 1) * NF],
                        op0=ALU.add, op1=ALU.add)
                    nc.sync.dma_start(
                        y_out.ap()[d * PC:(d + 1) * PC], yt[:])


def _get_nc():
    if "nc" not in _CACHE:
        _CACHE["nc"] = _build()
    return _CACHE["nc"]


def _prep_inputs(inputs):
    """Host-side slicing/transposition into per-core bf16 DRAM layouts."""
    f32 = np.float32
    x1 = np.ascontiguousarray(np.asarray(inputs["x1"], f32))
    x2 = np.ascontiguousarray(np.asarray(inputs["x2"], f32))
    Wq = np.asarray(inputs["Wq"], f32)
    Wkv = np.asarray(inputs["Wkv"], f32)
    Wqkv = np.asarray(inputs["Wqkv"], f32)
    W1 = np.asarray(inputs["W1"], f32)
    W2 = np.asarray(inputs["W2"], f32)
    bq = np.asarray(inputs["bq"], f32)
    bkv = np.asarray(inputs["bkv"], f32)
    bqkv = np.asarray(inputs["bqkv"], f32)
    gamma = np.asarray(inputs["gamma"], f32)
    beta = np.asarray(inputs["beta"], f32)
    b1 = np.asarray(inputs["b1"], f32)
    b2 = np.asarray(inputs["b2"], f32)

    def wcols(Wslice):     # (1024, 512) -> (4, 128, 8, 128) bf16
        return np.ascontiguousarray(
            Wslice.reshape(KC, PC, 4, PC).transpose(2, 1, 0, 3)).astype(BF)

    def bcols(bslice, n):  # (n*128,) -> (128, n) fp32
        return np.ascontiguousarray(bslice.reshape(n, PC).T)

    w1h = np.ascontiguousarray(
        W1.reshape(KC, PC, FT, PC).transpose(2, 1, 0, 3)).astype(BF)
    w2h = np.ascontiguousarray(
        W2.reshape(FT, PC, KC, PC).transpose(2, 1, 0, 3)).astype(BF)
    b1h = bcols(b1, FT)
    b2h = bcols(b2, KC)
    gh = bcols(gamma, KC)
    bh = bcols(beta, KC)

    in_maps = []
    for core in range(8):
        b, hh = core // 2, core % 2
        lo = NF * hh
        x2t = np.ascontiguousarray(x2[b].T)
        x1t = np.ascontiguousarray(x1[b].T)
        x2own = np.ascontiguousarray(
            x2t[:, lo:lo + NF].reshape(KC, PC, NF).transpose(1, 0, 2)
            .reshape(PC, KC * NF)).astype(BF)
        in_maps.append({
            "x2t": x2t.astype(BF), "x1t": x1t.astype(BF), "x2own": x2own,
            "wq": wcols(Wq[:, lo:lo + NF]),
            "wk": wcols(Wkv[:, lo:lo + NF]),
            "wv": np.ascontiguousarray(
                Wkv[:, D + lo:D + lo + NF].reshape(KC, PC, NF)).astype(BF),
            "wq2": wcols(Wqkv[:, lo:lo + NF]),
            "wk2": wcols(Wqkv[:, D + lo:D + lo + NF]),
            "wv2": np.ascontiguousarray(
                Wqkv[:, 2 * D + lo:2 * D + lo + NF]
                .reshape(KC, PC, NF)).astype(BF),
            "w1": w1h, "w2": w2h,
            "bqc": bcols(bq[lo:lo + NF], 4),
            "bkc": bcols(bkv[lo:lo + NF], 4),
            "bq2c": bcols(bqkv[lo:lo + NF], 4),
            "bk2c": bcols(bqkv[D + lo:D + lo + NF], 4),
            "bvr": np.ascontiguousarray(
                bkv[D + lo:D + lo + NF].reshape(1, NF)),
            "bv2r": np.ascontiguousarray(
                bqkv[2 * D + lo:2 * D + lo + NF].reshape(1, NF)),
            "b1c": b1h, "b2c": b2h, "gammac": gh, "betac": bh,
        })
    return in_maps


def kernel(**inputs):
    in_maps = _prep_inputs(inputs)
    nc = _get_nc()
    res = run_bass_kernel_spmd(nc, in_maps, core_ids=list(range(8)))
    _CACHE["last_results"] = res
    out = np.zeros((B, N, D), np.float32)
    for core in range(8):
        b, hh = core // 2, core % 2
        out[b, NF * hh:NF * hh + NF, :] = res.results[core]["y"].T
    return out



# revision 28
# speedup vs baseline: 1.1196x; 1.1196x over previous
"""Trainium2 Bass kernel for nn_CrossSelfDecoder (B=4,N=1024,D=1024,H=16,F=4096).

Sharding: 8 cores = (batch b in 0..3) x (head-half hh in 0..1). Each core
computes attention for its 8 heads over all 1024 positions of its batch.
Because the reference reshapes (B,H,N,Dp)->(B,N,D) without permuting heads
back, head-ownership makes row-ownership invariant: core (b,hh) owns rows
[512*hh, 512*hh+512) of batch b through the whole network.

Design:
- Host pre-transposes x1/x2 and pre-tiles all weights into bf16 DRAM
  layouts: zero device-side transposes, contiguous per-partition DMAs.
- All matmuls bf16 x bf16 with fp32 PSUM accumulate (PE at full rate).
- Attention: scores per (head, query-half) into [128,1024] PSUM tiles,
  exp merged to FD=1024 ACT calls, AV with a ones-column in V (M=65) so
  the softmax denominator falls out of the same matmul; denominator
  reciprocal via reciprocal_approx_fast + gpsimd partition_broadcast;
  the normalized output lands in the j-major transposed layout with 2
  coarse 4D-AP DVE ops per pair (no per-query scatter).
- LayerNorm after each attention (stats via 1/D-scaled ones matmuls,
  sqrt on ACT preloaded during the attention tail, rstd via
  reciprocal_approx_fast, gpsimd row broadcasts).
- One pairwise AllGather of the LN1 rows (bf16) overlapped with the
  own-row halves of the k2/v2 projections: keys are used in
  [own | partner] arrival order (softmax is key-permutation
  invariant), partner rows are recovered exactly as
  (block0 - own) + block1, and q2 reads the gathered blocks in global
  row order.
"""

import numpy as np
import ml_dtypes

import concourse.mybir as mybir
import concourse.tile as tile
from concourse import bacc
from concourse.bass_utils import run_bass_kernel_spmd

FP32 = mybir.dt.float32
BF16 = mybir.dt.bfloat16
AF = mybir.ActivationFunctionType
ALU = mybir.AluOpType

B, N, D, H, F = 4, 1024, 1024, 16, 4096
Dp = D // H           # 64
HPC = 8               # heads per core
PC = 128              # partition chunk
NF = 512              # free chunk (one psum bank of fp32)
KC = D // PC          # 8 contraction chunks
FT = F // PC          # 32 f-tiles
NB = 4                # LN/collective row bands of 128
EPS = 1e-5
BF = ml_dtypes.bfloat16

_CACHE = {}


def _build():
    nc = bacc.Bacc("TRN2", target_bir_lowering=False, debug=False,
                   num_devices=8)
    dram = {}
    specs = [
        ("x2t", [D, N], BF16), ("x1t", [D, N], BF16),
        ("x2own", [PC, KC * NF], BF16),
        ("wq", [4, PC, KC, PC], BF16), ("wk", [4, PC, KC, PC], BF16),
        ("wv", [KC, PC, NF], BF16),
        ("wq2", [4, PC, KC, PC], BF16), ("wk2", [4, PC, KC, PC], BF16),
        ("wv2", [KC, PC, NF], BF16),
        ("w1", [FT, PC, KC, PC], BF16), ("w2", [KC, PC, FT, PC], BF16),
        ("bqc", [PC, 4], FP32), ("bkc", [PC, 4], FP32),
        ("bq2c", [PC, 4], FP32), ("bk2c", [PC, 4], FP32),
        ("bvr", [1, NF], FP32), ("bv2r", [1, NF], FP32),
        ("b1c", [PC, FT], FP32), ("b2c", [PC, KC], FP32),
        ("gammac", [PC, KC], FP32), ("betac", [PC, KC], FP32),
    ]
    for nm, shp, dt in specs:
        dram[nm] = nc.dram_tensor(nm, shp, dt, kind="ExternalInput")
    y_out = nc.dram_tensor("y", [D, NF], FP32, kind="ExternalOutput")

    # collective staging: own LN1 rows out, both group blocks back
    ag_in = nc.dram_tensor("agin", [D, NF], BF16, kind="Internal")
    ag_out = nc.dram_tensor("agout", [2, D, NF], BF16, kind="Internal")

    with tile.TileContext(nc) as tc:
        _emit(nc, tc, dram, ag_in, ag_out, y_out)
    nc.compile()
    return nc


def _proj_T(nc, sub, psp, w_dram, bias_cols, rhs_of, out_tiles, tag,
            nf_range=(0, 1)):
    """out[m][:, nf*512:...] = (W.T @ rhs + bias), T-domain."""
    for m in range(4):
        wt = sub.tile([PC, KC, PC], BF16, tag=f"w_{tag}", bufs=3,
                      name=f"w_{tag}{m}")
        nc.sync.dma_start(wt[:], w_dram.ap()[m])
        for nf in nf_range:
            ps = psp.tile([PC, NF], FP32, tag="proj", bufs=7,
                          name=f"proj_{tag}{m}_{nf}")
            for kc in range(KC):
                nc.tensor.matmul(ps[:], wt[:, kc, :], rhs_of(kc, nf),
                                 start=(kc == 0), stop=(kc == KC - 1))
            nc.scalar.activation(out_tiles[m][:, nf * NF:(nf + 1) * NF],
                                 ps[:], AF.Identity,
                                 bias=bias_cols[:, m:m + 1])


def _proj_v(nc, sub, psp, w_dram, bvB, rhs_of, v_tiles, tag,
            pc_range=(0, 8)):
    """v natural (keys x 512 own-head cols) + per-head ones column.
    v_tiles: 8 x (128, 520): head h data cols [65h,65h+64), col 65h+64=1."""
    wts = []
    for kc in range(KC):
        wt = sub.tile([PC, NF], BF16, tag=f"wv_{tag}", bufs=KC,
                      name=f"wv_{tag}{kc}")
        nc.sync.dma_start(wt[:], w_dram.ap()[kc])
        wts.append(wt)
    for pc in range(*pc_range):
        ps = psp.tile([PC, NF], FP32, tag="proj", bufs=7, name=f"v_{tag}{pc}")
        for kc in range(KC):
            nc.tensor.matmul(ps[:], rhs_of(kc, pc), wts[kc][:],
                             start=(kc == 0), stop=(kc == KC - 1))
        vt3 = v_tiles[pc][:].rearrange("p (h c) -> p h c", h=HPC)
        ps3 = ps[:].rearrange("p (h c) -> p h c", h=HPC)
        bb3 = bvB[:].rearrange("p (h c) -> p h c", h=HPC)
        nc.vector.tensor_tensor(vt3[:, :, 0:Dp], ps3, bb3, op=ALU.add)


def _attention(nc, tc, sub, psp, qT, kT, v_tiles, xT, tag,
               after_group=None):
    """Own-head attention; writes normalized output into xT [128, 4096]
    (j-major T-domain). Query columns are in natural order; the output
    of query n lands at partition 64*(n%2)+dp of j-block (n%16)//2 at
    free offset 64*hloc + (n%512)//16. after_group(g) is invoked after
    the two heads covering xT row band [128g, 128g+128) are emitted."""
    for hloc in range(HPC):
        t4, r64 = hloc // 2, Dp * (hloc % 2)
        for qh in range(2):
            pts = []
            for g in range(4):          # kc groups of 2
                sps = psp.tile([PC, 2 * NF], FP32, tag="S", bufs=2,
                               name=f"S_{tag}{hloc}_{qh}_{g}")
                for k2 in range(2):
                    kc = 2 * g + k2
                    nc.tensor.matmul(
                        sps[:, k2 * NF:(k2 + 1) * NF],
                        kT[t4][r64:r64 + Dp, kc * PC:(kc + 1) * PC],
                        qT[t4][r64:r64 + Dp, qh * NF:(qh + 1) * NF],
                        start=True, stop=True)
                pt = sub.tile([PC, 2 * NF], BF16, tag="PT", bufs=8,
                              name=f"PT_{tag}{hloc}_{qh}_{g}")
                nc.scalar.activation(pt[:], sps[:], AF.Exp)
                pts.append(pt)
            ops = psp.tile([Dp + 1, NF], FP32, tag="O", bufs=4,
                           name=f"O_{tag}{hloc}_{qh}")
            for kc in range(KC):
                nc.tensor.matmul(
                    ops[:], v_tiles[kc][:, 65 * hloc:65 * hloc + 65],
                    pts[kc // 2][:, (kc % 2) * NF:(kc % 2 + 1) * NF],
                    start=(kc == 0), stop=(kc == KC - 1))
            drow = sub.tile([1, NF], FP32, tag="drow", bufs=4,
                            name=f"dr_{tag}{hloc}_{qh}")
            nc.vector.tensor_copy(drow[:], ops[Dp:Dp + 1, :])
            rrow = sub.tile([1, NF], FP32, tag="rrow", bufs=4,
                            name=f"rr_{tag}{hloc}_{qh}")
            nc.vector.reciprocal_approx_fast(rrow[:], drow[:])
            rb = sub.tile([Dp, NF], FP32, tag="rb", bufs=4,
                          name=f"rb_{tag}{hloc}_{qh}")
            nc.gpsimd.partition_broadcast(rb[:], rrow[:])
            # normalized scatter, 2 coarse ops (mm = n%2):
            # src col (within qh half) = 128wa + 16wb + 2j + mm
            # dst free = j*512 + 64hloc + 32qh + 8wa + wb
            toff = 8 * hloc + 4 * qh
            dst4 = xT.rearrange("p (j t wb) -> p j t wb", j=8, t=64, wb=8)
            for mm in range(2):
                dst = dst4[Dp * mm:Dp * mm + Dp, :, toff:toff + 4, :]
                src = ops[0:Dp, :].rearrange(
                    "d (wa wb j m) -> d m j wa wb",
                    wa=4, wb=8, j=8, m=2)[:, mm]
                srb = rb[:].rearrange(
                    "d (wa wb j m) -> d m j wa wb",
                    wa=4, wb=8, j=8, m=2)[:, mm]
                nc.vector.tensor_tensor(dst, src, srb, op=ALU.mult)
        if after_group is not None and hloc % 2 == 1:
            after_group(hloc // 2)


def _ln_full(nc, tc, sub, xT, write_out, c, tag, warm_n=40):
    """LayerNorm of all 512 rows of xT [128, 4096] over the feature
    axis. write_out(j, src_tile) stores the j-th [128, 512] result."""
    with tc.tile_pool(name=f"ln_{tag}", space="PSUM", bufs=1) as psp:
        lnr = psp.tile([33, NF], FP32, tag="lnr", bufs=1,
                       name=f"lnr_{tag}")
        sq = None
        for j in range(KC):
            xj = xT[:, j * NF:(j + 1) * NF]
            nc.tensor.matmul(lnr[0:1, :], c["onesd"][:], xj,
                             start=(j == 0), stop=(j == KC - 1))
            sq = sub.tile([PC, NF], BF16, tag="sq", bufs=3,
                          name=f"sq_{tag}{j}")
            nc.vector.tensor_tensor(sq[:], xj, xj, op=ALU.mult)
            nc.tensor.matmul(lnr[32:33, :], c["onesd"][:], sq[:],
                             start=(j == 0), stop=(j == KC - 1))
        # keep the PE clock-gate warm through the rows/apply window
        wps = psp.tile([1, NF], FP32, tag="lnwarm", bufs=1,
                       name=f"lnwarm_{tag}")
        for i in range(warm_n):
            nc.tensor.matmul(wps[:], c["onesd"][:], sq[:],
                             start=(i == 0), stop=(i == warm_n - 1))
        # preload the sqrt table while the attention tail drains
        wsq = sub.tile([1, 8], FP32, tag="lrow", bufs=8, name=f"wsq_{tag}")
        nc.gpsimd.memset(wsq[:], 1.0)
        nc.scalar.activation(wsq[:], wsq[:], AF.Sqrt)
        mu = sub.tile([1, NF], FP32, tag="lrow", bufs=8, name=f"mu_{tag}")
        nc.vector.tensor_copy(mu[:], lnr[0:1, :])
        mub = sub.tile([PC, NF], FP32, tag="lnb", bufs=2, name=f"mub_{tag}")
        nc.gpsimd.partition_broadcast(mub[:], mu[:])
        mu2 = sub.tile([1, NF], FP32, tag="lrow", bufs=8, name=f"mu2_{tag}")
        nc.vector.tensor_tensor(mu2[:], mu[:], mu[:], op=ALU.mult)
        var = sub.tile([1, NF], FP32, tag="lrow", bufs=8, name=f"var_{tag}")
        nc.vector.tensor_tensor(var[:], lnr[32:33, :], mu2[:],
                                op=ALU.subtract)
        std = sub.tile([1, NF], FP32, tag="lrow", bufs=8, name=f"std_{tag}")
        nc.scalar.activation(std[:], var[:], AF.Sqrt, bias=c["eps_sc"][:])
        rstd = sub.tile([1, NF], FP32, tag="lrow", bufs=8,
                        name=f"rstd_{tag}")
        nc.vector.reciprocal_approx_fast(rstd[:], std[:])
        rstdb = sub.tile([PC, NF], FP32, tag="lnb", bufs=2,
                         name=f"rsb_{tag}")
        nc.gpsimd.partition_broadcast(rstdb[:], rstd[:])
        for j in range(KC):
            xj = xT[:, j * NF:(j + 1) * NF]
            t1 = sub.tile([PC, NF], BF16, tag="lntmp", bufs=3,
                          name=f"lt_{tag}{j}")
            nc.vector.tensor_tensor(t1[:], xj, mub[:], op=ALU.subtract)
            t2 = sub.tile([PC, NF], BF16, tag="lntmp2", bufs=3,
                          name=f"l2_{tag}{j}")
            nc.vector.tensor_tensor(t2[:], t1[:], rstdb[:], op=ALU.mult)
            write_out(j, t2)


def _ln_band(nc, sub, psp, xT, g, write_out, c, tag):
    """LayerNorm of xT row band [128g, 128g+128) (local rows), over the
    feature axis (partitions x 8 j-blocks). write_out(j, src_tile)
    stores the [128, 128] result for j-block j."""
    lnr = psp.tile([33, PC], FP32, tag="lnr", bufs=2, name=f"lnr_{tag}{g}")
    s0, s1 = lnr[0:1, :], lnr[32:33, :]
    for j in range(KC):
        xj = xT[:, j * NF + PC * g:j * NF + PC * g + PC]
        nc.tensor.matmul(s0, c["onesd"][:], xj,
                         start=(j == 0), stop=(j == KC - 1))
        sq = sub.tile([PC, PC], BF16, tag="sq", bufs=4,
                      name=f"sq_{tag}{g}_{j}")
        nc.vector.tensor_tensor(sq[:], xj, xj, op=ALU.mult)
        nc.tensor.matmul(s1, c["onesd"][:], sq[:],
                         start=(j == 0), stop=(j == KC - 1))
    # s0 = mean, s1 = E[x^2] (stat matmul ones are pre-scaled by 1/D)
    mu = sub.tile([1, PC], FP32, tag="lrow", bufs=8, name=f"mu_{tag}{g}")
    nc.vector.tensor_copy(mu[:], s0)
    mu2 = sub.tile([1, PC], FP32, tag="lrow", bufs=8, name=f"mu2_{tag}{g}")
    nc.vector.tensor_tensor(mu2[:], mu[:], mu[:], op=ALU.mult)
    var = sub.tile([1, PC], FP32, tag="lrow", bufs=8, name=f"var_{tag}{g}")
    nc.vector.tensor_tensor(var[:], s1, mu2[:], op=ALU.subtract)
    std = sub.tile([1, PC], FP32, tag="lrow", bufs=8, name=f"std_{tag}{g}")
    nc.scalar.activation(std[:], var[:], AF.Sqrt, bias=c["eps_sc"][:])
    rstd = sub.tile([1, PC], FP32, tag="lrow", bufs=8, name=f"rstd_{tag}{g}")
    nc.vector.reciprocal_approx_fast(rstd[:], std[:])
    mub = sub.tile([PC, PC], FP32, tag="lnb", bufs=4, name=f"mub_{tag}{g}")
    nc.gpsimd.partition_broadcast(mub[:], mu[:])
    rstdb = sub.tile([PC, PC], FP32, tag="lnb", bufs=4, name=f"rsb_{tag}{g}")
    nc.gpsimd.partition_broadcast(rstdb[:], rstd[:])
    for j in range(KC):
        xj = xT[:, j * NF + PC * g:j * NF + PC * g + PC]
        t1 = sub.tile([PC, PC], BF16, tag="lntmp", bufs=3,
                      name=f"lt_{tag}{g}_{j}")
        nc.vector.tensor_tensor(t1[:], xj, mub[:], op=ALU.subtract)
        t2 = sub.tile([PC, PC], BF16, tag="lntmp2", bufs=3,
                      name=f"l2_{tag}{g}_{j}")
        nc.vector.tensor_tensor(t2[:], t1[:], rstdb[:], op=ALU.mult)
        write_out(j, t2)


def _emit(nc, tc, dram, ag_in, ag_out, y_out):
    with tc.tile_pool(name="persist", bufs=1) as pp:
        def bias_tile(name):
            shp = list(dram[name].shape)
            return pp.tile(shp, FP32, tag=f"bt_{name}", name=f"bt_{name}")

        bias_names = ("bqc", "bkc", "bq2c", "bk2c", "b1c", "b2c",
                      "gammac", "betac")
        c = {}
        for nm in bias_names:
            c[nm] = bias_tile(nm)
        bvr = bias_tile("bvr")
        bv2r = bias_tile("bv2r")

        def load_biases():
            for nm in bias_names:
                nc.sync.dma_start(c[nm][:], dram[nm].ap())
            nc.sync.dma_start(bvr[:], dram["bvr"].ap())
            nc.sync.dma_start(bv2r[:], dram["bv2r"].ap())

        onesd = pp.tile([PC, 1], BF16, tag="onesd")
        nc.gpsimd.memset(onesd[:], 1.0 / D)
        c["onesd"] = onesd
        eps_sc = pp.tile([1, 1], FP32, tag="eps_sc")
        nc.gpsimd.memset(eps_sc[:], EPS)
        c["eps_sc"] = eps_sc

        bvB = pp.tile([PC, NF], FP32, tag="bvB")
        bv2B = pp.tile([PC, NF], FP32, tag="bv2B")

        # table warm-up: preload the exp set during initial DMAs
        warm = pp.tile([1, 8], FP32, tag="warm")
        nc.gpsimd.memset(warm[:], 1.0)
        nc.scalar.activation(warm[:], warm[:], AF.Exp)
        # PE warm-up: keep the HAM activity window busy while the first
        # input tiles stream in, so real matmuls start at full clock
        wmm = pp.tile([PC, NF], BF16, tag="wmm")
        nc.gpsimd.memset(wmm[:], 0.0)
        with tc.tile_pool(name="warmps", space="PSUM", bufs=1) as wps:
            wp = wps.tile([1, NF], FP32, tag="warmp", bufs=1)
            for i in range(40):
                nc.tensor.matmul(wp[:], onesd[:], wmm[:],
                                 start=(i == 0), stop=(i == 39))

        # cross-stage persistents
        nTo = pp.tile([PC, KC * NF], BF16, tag="nTo")
        n3T = pp.tile([PC, KC * NF], BF16, tag="n3T")

        # ---- stage 1 ----
        with tc.tile_pool(name="st1", bufs=1) as sub:
            x2own = sub.tile([PC, KC * NF], BF16, tag="x2own")
            qT = [sub.tile([PC, N], BF16, tag="qT", bufs=4, name=f"qT{i}")
                  for i in range(4)]
            kT = [sub.tile([PC, N], BF16, tag="kT", bufs=4, name=f"kT{i}")
                  for i in range(4)]
            vt = [sub.tile([PC, 65 * HPC], BF16, tag="vt", bufs=KC,
                           name=f"vt{i}") for i in range(KC)]
            xT = sub.tile([PC, KC * NF], BF16, tag="xT")

            with tc.tile_pool(name="s1x", bufs=1) as subx:
                x2T = [subx.tile([PC, N], BF16, tag="x2T", bufs=KC,
                                 name=f"x2T{i}") for i in range(KC)]
                for j in range(KC):
                    nc.sync.dma_start(
                        x2T[j][:], dram["x2t"].ap()[j * PC:(j + 1) * PC])
                load_biases()
                nc.gpsimd.partition_broadcast(bvB[:], bvr[:])
                nc.gpsimd.partition_broadcast(bv2B[:], bv2r[:])
                x1T = [subx.tile([PC, N], BF16, tag="x1T", bufs=KC,
                                 name=f"x1T{i}") for i in range(KC)]
                with tc.tile_pool(name="s1p", space="PSUM", bufs=1) as psp:
                    _proj_T(nc, subx, psp, dram["wq"], c["bqc"],
                            lambda kc, nf: x2T[kc][:, nf * NF:(nf + 1) * NF],
                            qT, "q")
                    for j in range(KC):
                        nc.sync.dma_start(
                            x1T[j][:],
                            dram["x1t"].ap()[j * PC:(j + 1) * PC])
                    nc.sync.dma_start(x2own[:], dram["x2own"].ap())
                    for i in range(KC):
                        v3 = vt[i][:].rearrange("p (h c) -> p h c", h=HPC)
                        nc.gpsimd.memset(v3[:, :, Dp:Dp + 1].squeeze(2), 1.0)
                    _proj_T(nc, subx, psp, dram["wk"], c["bkc"],
                            lambda kc, nf: x1T[kc][:, nf * NF:(nf + 1) * NF],
                            kT, "k")
                    _proj_v(nc, subx, psp, dram["wv"], bvB,
                            lambda kc, pc: x1T[kc][:, pc * PC:(pc + 1) * PC],
                            vt, "v1")

            with tc.tile_pool(name="s1a", space="PSUM", bufs=1) as psp:
                def after_group1(g):
                    # residual for band g (pure DVE, off the exp table)
                    bnd = xT[:].rearrange("p (j r) -> p j r", j=KC)[
                        :, :, PC * g:PC * g + PC]
                    x2b = x2own[:].rearrange("p (j r) -> p j r", j=KC)[
                        :, :, PC * g:PC * g + PC]
                    nc.vector.tensor_tensor(bnd, bnd, x2b, op=ALU.add)

                _attention(nc, tc, sub, psp, qT, kT, vt, xT[:], "x",
                           after_group=after_group1)

            def ln1_out(j, t2):
                nc.scalar.activation(
                    nTo[:, j * NF:(j + 1) * NF], t2[:], AF.Identity,
                    bias=c["betac"][:, j:j + 1],
                    scale=c["gammac"][:, j:j + 1])
                nc.sync.dma_start(ag_in.ap()[j * PC:(j + 1) * PC],
                                  nTo[:, j * NF:(j + 1) * NF])

            _ln_full(nc, tc, sub, xT[:], ln1_out, c, "ln1")
            nc.gpsimd.collective_compute(
                "AllGather", ALU.bypass,
                replica_groups=[[0, 1], [2, 3], [4, 5], [6, 7]],
                ins=[ag_in.ap()], outs=[ag_out.ap()])

        w1pre = [pp.tile([PC, KC, PC], BF16, tag="w1pre", bufs=8,
                         name=f"w1pre{i}") for i in range(8)]

        # ---- stage 2 ----
        # keys are used in arrival order [own rows | partner rows]
        # (softmax is key-permutation invariant); queries need global
        # order, which nTg (both gathered blocks) provides uniformly.
        with tc.tile_pool(name="st2", bufs=1) as sub:
            for f in range(8):
                nc.sync.dma_start(w1pre[f][:], dram["w1"].ap()[f])
            q2T = [sub.tile([PC, N], BF16, tag="q2T", bufs=4,
                            name=f"q2T{i}") for i in range(4)]
            k2T = [sub.tile([PC, N], BF16, tag="k2T", bufs=4,
                            name=f"k2T{i}") for i in range(4)]
            v2t = [sub.tile([PC, 65 * HPC], BF16, tag="v2t", bufs=KC,
                            name=f"v2t{i}") for i in range(KC)]
            x3T = sub.tile([PC, KC * NF], BF16, tag="x3T")
            for i in range(KC):
                v3 = v2t[i][:].rearrange("p (h c) -> p h c", h=HPC)
                nc.gpsimd.memset(v3[:, :, Dp:Dp + 1].squeeze(2), 1.0)

            with tc.tile_pool(name="s2p", space="PSUM", bufs=1) as psp:
                # own-row halves of k2/v2 run from nTo while the
                # AllGather is in flight
                _proj_v(nc, sub, psp, dram["wv2"], bv2B,
                        lambda kc, pc: nTo[:, kc * NF + pc * PC:
                                           kc * NF + (pc + 1) * PC],
                        v2t, "v2o", pc_range=(0, 4))
                k2w = [sub.tile([PC, KC, PC], BF16, tag="w_k2", bufs=4,
                                name=f"wk2_{m}") for m in range(4)]
                for m in range(4):
                    nc.sync.dma_start(k2w[m][:], dram["wk2"].ap()[m])
                for m in range(4):
                    ps = psp.tile([PC, NF], FP32, tag="proj", bufs=7,
                                  name=f"k2o_{m}")
                    for kc in range(KC):
                        nc.tensor.matmul(
                            ps[:], k2w[m][:, kc, :],
                            nTo[:, kc * NF:kc * NF + NF],
                            start=(kc == 0), stop=(kc == KC - 1))
                    nc.scalar.activation(k2T[m][:, 0:NF], ps[:],
                                         AF.Identity,
                                         bias=c["bk2c"][:, m:m + 1])

                wp2 = psp.tile([1, NF], FP32, tag="cwarm", bufs=1,
                               name="cwarm")
                for i in range(88):
                    nc.tensor.matmul(wp2[:], c["onesd"][:], k2T[0][:, 0:NF],
                                     start=(i == 0), stop=(i == 87))

                # gathered blocks (global row order) + exact partner
                # recovery: partner = (block0 - own) + block1
                nTg = [sub.tile([PC, N], BF16, tag="nTg", bufs=KC,
                                name=f"nTg{i}") for i in range(KC)]
                for j in range(KC):
                    for r in range(2):
                        nc.sync.dma_start(
                            nTg[j][:, r * NF:(r + 1) * NF],
                            ag_out.ap()[r, j * PC:(j + 1) * PC])
                nTp = [sub.tile([PC, NF], BF16, tag="nTp", bufs=KC,
                                name=f"nTp{i}") for i in range(KC)]
                for j in range(KC):
                    tdif = sub.tile([PC, NF], FP32, tag="tdif", bufs=4,
                                    name=f"tdif{j}")
                    nc.vector.tensor_tensor(
                        tdif[:], nTg[j][:, 0:NF],
                        nTo[:, j * NF:(j + 1) * NF], op=ALU.subtract)
                    nc.vector.tensor_tensor(
                        nTp[j][:], tdif[:], nTg[j][:, NF:N], op=ALU.add)

                _proj_v(nc, sub, psp, dram["wv2"], bv2B,
                        lambda kc, pc: nTp[kc][:, (pc - 4) * PC:
                                               (pc - 3) * PC],
                        v2t, "v2p", pc_range=(4, 8))
                for m in range(4):
                    ps = psp.tile([PC, NF], FP32, tag="proj", bufs=7,
                                  name=f"k2p_{m}")
                    for kc in range(KC):
                        nc.tensor.matmul(
                            ps[:], k2w[m][:, kc, :], nTp[kc][:],
                            start=(kc == 0), stop=(kc == KC - 1))
                    nc.scalar.activation(k2T[m][:, NF:N], ps[:],
                                         AF.Identity,
                                         bias=c["bk2c"][:, m:m + 1])
                _proj_T(nc, sub, psp, dram["wq2"], c["bq2c"],
                        lambda kc, nf: nTg[kc][:, nf * NF:(nf + 1) * NF],
                        q2T, "q2")

            with tc.tile_pool(name="s2a", space="PSUM", bufs=1) as psp:
                def after_group2(g):
                    bnd = x3T[:].rearrange("p (j r) -> p j r", j=KC)[
                        :, :, PC * g:PC * g + PC]
                    nob = nTo[:].rearrange("p (j r) -> p j r", j=KC)[
                        :, :, PC * g:PC * g + PC]
                    nc.vector.tensor_tensor(bnd, bnd, nob, op=ALU.add)

                _attention(nc, tc, sub, psp, q2T, k2T, v2t, x3T[:], "y",
                           after_group=after_group2)

            def ln2_out(j, t2):
                nc.scalar.activation(
                    n3T[:, j * NF:(j + 1) * NF], t2[:], AF.Identity,
                    bias=c["betac"][:, j:j + 1],
                    scale=c["gammac"][:, j:j + 1])

            _ln_full(nc, tc, sub, x3T[:], ln2_out, c, "ln2")

        # ---- stage 3: MLP ----
        with tc.tile_pool(name="s3", bufs=1) as sub:
            hT = [sub.tile([PC, NF], BF16, tag="hT", bufs=FT,
                           name=f"hT{i}") for i in range(FT)]
            with tc.tile_pool(name="s3p", space="PSUM", bufs=1) as psp:
                for f in range(FT):
                    if f < 8:
                        wt = w1pre[f]
                    else:
                        wt = sub.tile([PC, KC, PC], BF16, tag="w1t", bufs=4,
                                      name=f"w1t{f}")
                        nc.sync.dma_start(wt[:], dram["w1"].ap()[f])
                    ps = psp.tile([PC, NF], FP32, tag="mlp", bufs=8,
                                  name=f"h{f}")
                    for kc in range(KC):
                        nc.tensor.matmul(
                            ps[:], wt[:, kc, :],
                            n3T[:, kc * NF:(kc + 1) * NF],
                            start=(kc == 0), stop=(kc == KC - 1))
                    nc.scalar.activation(hT[f][:], ps[:], AF.Gelu,
                                         bias=c["b1c"][:, f:f + 1])
                for d in range(KC):
                    w2t = sub.tile([PC, FT, PC], BF16, tag="w2t", bufs=2,
                                   name=f"w2t{d}")
                    nc.sync.dma_start(w2t[:], dram["w2"].ap()[d])
                    ps = psp.tile([PC, NF], FP32, tag="mlp", bufs=8,
                                  name=f"yp{d}")
                    for f in range(FT):
                        nc.tensor.matmul(ps[:], w2t[:, f, :], hT[f][:],
                                         start=(f == 0), stop=(f == FT - 1))
                    yt = sub.tile([PC, NF], FP32, tag="yT", bufs=4,
                                  name=f"yT{d}")
                    nc.vector.scalar_tensor_tensor(
                        yt[:], ps[:], c["b2c"][:, d:d + 1],
                        n3T[:, d * NF:(d + 1) * NF],
                        op0=ALU.add, op1=ALU.add)
                    nc.sync.dma_start(
                        y_out.ap()[d * PC:(d + 1) * PC], yt[:])


def _get_nc():
    if "nc" not in _CACHE:
        _CACHE["nc"] = _build()
    return _CACHE["nc"]


def _prep_inputs(inputs):
    """Host-side slicing/transposition into per-core bf16 DRAM layouts."""
    f32 = np.float32
    x1 = np.ascontiguousarray(np.asarray(inputs["x1"], f32))
    x2 = np.ascontiguousarray(np.asarray(inputs["x2"], f32))
    Wq = np.asarray(inputs["Wq"], f32)
    Wkv = np.asarray(inputs["Wkv"], f32)
    Wqkv = np.asarray(inputs["Wqkv"], f32)
    W1 = np.asarray(inputs["W1"], f32)
    W2 = np.asarray(inputs["W2"], f32)
    bq = np.asarray(inputs["bq"], f32)
    bkv = np.asarray(inputs["bkv"], f32)
    bqkv = np.asarray(inputs["bqkv"], f32)
    gamma = np.asarray(inputs["gamma"], f32)
    beta = np.asarray(inputs["beta"], f32)
    b1 = np.asarray(inputs["b1"], f32)
    b2 = np.asarray(inputs["b2"], f32)

    def wcols(Wslice):     # (1024, 512) -> (4, 128, 8, 128) bf16
        return np.ascontiguousarray(
            Wslice.reshape(KC, PC, 4, PC).transpose(2, 1, 0, 3)).astype(BF)

    def bcols(bslice, n):  # (n*128,) -> (128, n) fp32
        return np.ascontiguousarray(bslice.reshape(n, PC).T)

    w1h = np.ascontiguousarray(
        W1.reshape(KC, PC, FT, PC).transpose(2, 1, 0, 3)).astype(BF)
    w2h = np.ascontiguousarray(
        W2.reshape(FT, PC, KC, PC).transpose(2, 1, 0, 3)).astype(BF)
    b1h = bcols(b1, FT)
    b2h = bcols(b2, KC)
    gh = bcols(gamma, KC)
    bh = bcols(beta, KC)

    in_maps = []
    for core in range(8):
        b, hh = core // 2, core % 2
        lo = NF * hh
        x2t = np.ascontiguousarray(x2[b].T)
        x1t = np.ascontiguousarray(x1[b].T)
        x2own = np.ascontiguousarray(
            x2t[:, lo:lo + NF].reshape(KC, PC, NF).transpose(1, 0, 2)
            .reshape(PC, KC * NF)).astype(BF)
        in_maps.append({
            "x2t": x2t.astype(BF), "x1t": x1t.astype(BF), "x2own": x2own,
            "wq": wcols(Wq[:, lo:lo + NF]),
            "wk": wcols(Wkv[:, lo:lo + NF]),
            "wv": np.ascontiguousarray(
                Wkv[:, D + lo:D + lo + NF].reshape(KC, PC, NF)).astype(BF),
            "wq2": wcols(Wqkv[:, lo:lo + NF]),
            "wk2": wcols(Wqkv[:, D + lo:D + lo + NF]),
            "wv2": np.ascontiguousarray(
                Wqkv[:, 2 * D + lo:2 * D + lo + NF]
                .reshape(KC, PC, NF)).astype(BF),
            "w1": w1h, "w2": w2h,
            "bqc": bcols(bq[lo:lo + NF], 4),
            "bkc": bcols(bkv[lo:lo + NF], 4),
            "bq2c": bcols(bqkv[lo:lo + NF], 4),
            "bk2c": bcols(bqkv[D + lo:D + lo + NF], 4),
            "bvr": np.ascontiguousarray(
                bkv[D + lo:D + lo + NF].reshape(1, NF)),
            "bv2r": np.ascontiguousarray(
                bqkv[2 * D + lo:2 * D + lo + NF].reshape(1, NF)),
            "b1c": b1h, "b2c": b2h, "gammac": gh, "betac": bh,
        })
    return in_maps


def kernel(**inputs):
    in_maps = _prep_inputs(inputs)
    nc = _get_nc()
    res = run_bass_kernel_spmd(nc, in_maps, core_ids=list(range(8)))
    _CACHE["last_results"] = res
    out = np.zeros((B, N, D), np.float32)
    for core in range(8):
        b, hh = core // 2, core % 2
        out[b, NF * hh:NF * hh + NF, :] = res.results[core]["y"].T
    return out

